# revision 47
# baseline (speedup 1.0000x reference)
"""Trainium2 Bass kernel for nn_AttentionModel (sparse_attention).

Reference computation:
    x = emb_table[tokens]                  # [B,S,D]
    scores = x @ x^T per batch             # [B,S,S]
    out = softmax(scores) @ x              # [B,S,D]
    logits = out[:, 0, :] @ cls_w.T + cls_b

Only row 0 of the attention output is used, so per batch element we only
need attention row 0:
    q = x[0]; s = X q; a = exp(s); logits = (a^T X / sum(a)) cls_w^T + cls_b
This turns ~275 GFLOP of full attention into an embedding gather plus
~4 MFLOP per batch element.

Sharding: data-parallel over batch. 8 cores x 4 batch elements; emb_table
and classifier weights replicated; no collectives.
"""

import numpy as np

import bass_rust

import concourse.bass as bass
import concourse.mybir as mybir
import concourse.tile as tile
from concourse.bass_utils import run_bass_kernel_spmd


def _split_multiwaits(nc: bass.Bass) -> None:
    """Workaround for the walrus build in this container, which rejects
    instructions carrying more than one sync-wait command ("Too many sync
    wait commands" / "ISA wrong length" in CoreV3GenImpl setupSyncWait).

    Moves each instruction's sync waits onto dedicated single-wait NOPs
    inserted right before it on the same engine stream (bass_nofuse so
    walrus's nop-fusion can't merge them back). Semantics are identical:
    the engine's sequencer blocks on the NOPs' waits, then issues the
    now-waitless instruction. Sem updates stay on the original."""
    counter = 0
    fn = nc.m.functions[0]
    for bb in fn.blocks:
        insts = bb.instructions
        new_list = []
        changed = False
        for inst in insts:
            si = inst.sync_info
            waits = list(si.on_wait) if si is not None else []
            if waits:
                for w in waits:
                    counter += 1
                    new_list.append(
                        mybir.InstNoOp(
                            name=f"waitnop-{counter}",
                            engine=inst.engine,
                            ins=[],
                            outs=[],
                            bass_nofuse=True,
                            sync_info=bass_rust.SyncInfo(on_wait=[w], on_update=[]),
                        )
                    )
                inst.sync_info = bass_rust.SyncInfo(
                    on_wait=[], on_update=list(si.on_update)
                )
                changed = True
            new_list.append(inst)
        if changed:
            bb.instructions = new_list

B, S, D, V, C = 32, 2048, 512, 32000, 2
N_CORES = 8
BPC = B // N_CORES          # batch elements per core
SBLK = S // 128             # 16 free-dim blocks of gathered tokens

F32 = mybir.dt.float32
BF16 = mybir.dt.bfloat16
I32 = mybir.dt.int32

_CACHE: dict = {}


def _build_nc(split: bool = True) -> bass.Bass:
    nc = bass.Bass()
    emb_d = nc.dram_tensor("emb", [V, D], BF16, kind="ExternalInput")
    tok_d = nc.dram_tensor("tok", [128, BPC * SBLK], I32, kind="ExternalInput")
    cw_d = nc.dram_tensor("cls_w", [C, D], BF16, kind="ExternalInput")
    cb_d = nc.dram_tensor("cls_b", [C, 1], F32, kind="ExternalInput")
    out_d = nc.dram_tensor("out", [BPC, C], F32, kind="ExternalOutput")

    mult = mybir.AluOpType.mult
    add = mybir.AluOpType.add
    EXP = mybir.ActivationFunctionType.Exp

    with tile.TileContext(nc) as tc:
        with (
            tc.tile_pool(name="const", bufs=1) as constp,
            tc.tile_pool(name="xp", bufs=BPC) as xp,
            tc.tile_pool(name="sp", bufs=2) as sp,
            tc.tile_pool(name="jp", bufs=2) as jp,
            tc.tile_pool(name="tp", bufs=4) as tp,
            tc.tile_pool(name="ps", bufs=2, space="PSUM") as pp,
        ):
            idx0 = constp.tile([128, 1], I32)
            nc.sync.dma_start(idx0[:, :], tok_d[:, 0:1])
            idx = constp.tile([128, BPC, SBLK], I32)
            nc.sync.dma_start(idx[:, :, :], tok_d[:, :])
            ones1 = constp.tile([1, 128], BF16)
            nc.vector.memset(ones1[:], 1.0)
            ones128 = constp.tile([128, 1], F32)
            nc.vector.memset(ones128[:], 1.0)
            cw = constp.tile([C, D], BF16)
            nc.sync.dma_start(cw[:], cw_d[:, :])
            cb = constp.tile([C, 1], F32)
            nc.sync.dma_start(cb[:], cb_d[:, :])
            for b in range(BPC):
                # Gather the 2048 embedding rows for this batch element.
                # Token t lands on partition t%128, free block t//128; one
                # indirect DMA per 128-token block (one index per partition).
                x = xp.tile([128, SBLK, D], BF16, tag="x")
                for j in range(SBLK):
                    iap = (idx0[:, 0:1] if (b == 0 and j == 0)
                           else idx[:, b, j : j + 1])
                    nc.gpsimd.indirect_dma_start(
                        out=x[:, j, :],
                        out_offset=None,
                        in_=emb_d[:, :],
                        in_offset=bass.IndirectOffsetOnAxis(ap=iap, axis=0),
                    )

                # Broadcast q = x[token 0] to all 128 partitions via a K=1
                # outer-product matmul: ones[1,128]^T @ x[0:1, 0, :].
                qb = pp.tile([128, D], F32, tag="qb")
                nc.tensor.matmul(qb[:], ones1[:], x[0:1, 0, :], start=True, stop=True)
                qbs = sp.tile([128, D], BF16, tag="qbs")
                nc.scalar.copy(qbs[:], qb[:])

                # Scores s[t] = <x_t, q>: fused multiply+reduce per block
                # ((x*1) * q with accum_out = row sums).
                s = sp.tile([128, SBLK], F32, tag="s")
                for j in range(SBLK):
                    junk = jp.tile([128, D], BF16, tag="junk")
                    nc.vector.scalar_tensor_tensor(
                        out=junk[:],
                        in0=x[:, j, :],
                        scalar=1.0,
                        in1=qbs[:],
                        op0=mult,
                        op1=mult,
                        accum_out=s[:, j : j + 1],
                    )

                # a = exp(s) (scores are O(0.2): no max subtraction needed),
                # with fused per-partition row sums for the softmax denom.
                # Split into groups of 4 blocks so the pooled chain becomes
                # ready incrementally (tracks the gather stream, keeps PE
                # warm, and shrinks the end-of-kernel tail to one group).
                EG = 16
                e = sp.tile([128, SBLK], BF16, tag="e")
                zcols = sp.tile([128, EG], F32, tag="zcols")
                for g in range(EG):
                    lo, hi = g * (SBLK // EG), (g + 1) * (SBLK // EG)
                    nc.scalar.activation(
                        e[:, lo:hi], s[:, lo:hi], EXP,
                        accum_out=zcols[:, g : g + 1],
                    )

                # pooled = a^T X (unnormalized), accumulated over blocks,
                # replicated to C partitions via lhsT free-dim broadcast.
                pooled = pp.tile([C, D], F32, tag="pooled")
                for j in range(SBLK):
                    nc.tensor.matmul(
                        pooled[:],
                        e[:, j : j + 1].broadcast_to([128, C]),
                        x[:, j, :],
                        start=(j == 0),
                        stop=(j == SBLK - 1),
                    )

                # Z = sum over partitions of all zcol groups, C-replicated.
                zps = pp.tile([C, 1], F32, tag="z")
                for g in range(EG):
                    nc.tensor.matmul(
                        zps[:], zcols[:, g : g + 1].broadcast_to([128, C]),
                        ones128[:, :],
                        start=(g == 0), stop=(g == EG - 1),
                    )

                rz = tp.tile([C, 1], F32, tag="rz")
                nc.vector.reciprocal(rz[:], zps[:])

                # logits_c = <pooled, cls_w_c>: one fused dot over C rows,
                # reading pooled directly from PSUM (skips the sbuf copy)
                junk2 = tp.tile([C, D], BF16, tag="junk2")
                lg = tp.tile([C, 1], F32, tag="lg")
                nc.vector.scalar_tensor_tensor(
                    out=junk2[:],
                    in0=pooled[:],
                    scalar=1.0,
                    in1=cw[:],
                    op0=mult,
                    op1=mult,
                    accum_out=lg[:],
                )

                # out = lg / Z + cls_b
                ob = tp.tile([C, 1], F32, tag="ob")
                nc.vector.scalar_tensor_tensor(
                    ob[:], lg[:], rz[:], cb[:], op0=mult, op1=add
                )
                nc.sync.dma_start(out_d[b, :].unsqueeze(1), ob[:, :])

    nc.finalize()
    if split:
        _split_multiwaits(nc)
    return nc


def _wrap_tokens(tokens_row: np.ndarray) -> np.ndarray:
    """[S] int tokens -> [128, SBLK] int32; token t at [t%128, t//128]."""
    return np.ascontiguousarray(tokens_row.reshape(SBLK, 128).T.astype(np.int32))


def get_nc() -> bass.Bass:
    if "nc" not in _CACHE:
        _CACHE["nc"] = _build_nc()
    return _CACHE["nc"]


def make_in_maps(tokens, emb_table, cls_w, cls_b):
    import ml_dtypes

    tokens = np.asarray(tokens)
    emb = np.ascontiguousarray(
        np.asarray(emb_table, dtype=np.float32).astype(ml_dtypes.bfloat16)
    )
    cw = np.ascontiguousarray(
        np.asarray(cls_w, dtype=np.float32).astype(ml_dtypes.bfloat16)
    )
    cb = np.ascontiguousarray(np.asarray(cls_b, dtype=np.float32).reshape(C, 1))
    in_maps = []
    for core in range(N_CORES):
        idx_flat = np.concatenate(
            [_wrap_tokens(tokens[core * BPC + b]) for b in range(BPC)], axis=1
        )  # [128, BPC*SBLK]
        in_maps.append(
            {
                "emb": emb,
                "tok": np.ascontiguousarray(idx_flat),
                "cls_w": cw,
                "cls_b": cb,
            }
        )
    return in_maps


def kernel(tokens, emb_table, cls_w, cls_b) -> np.ndarray:
    nc = get_nc()
    in_maps = make_in_maps(tokens, emb_table, cls_w, cls_b)
    res = run_bass_kernel_spmd(nc, in_maps, core_ids=list(range(N_CORES)))
    outs = [res.results[c]["out"] for c in range(N_CORES)]
    return np.concatenate(outs, axis=0).astype(np.float32)



# revision 48
# speedup vs baseline: 1.0071x; 1.0071x over previous
"""Trainium2 Bass kernel for nn_AttentionModel (sparse_attention).

Reference computation:
    x = emb_table[tokens]                  # [B,S,D]
    scores = x @ x^T per batch             # [B,S,S]
    out = softmax(scores) @ x              # [B,S,D]
    logits = out[:, 0, :] @ cls_w.T + cls_b

Only row 0 of the attention output is used, so per batch element we only
need attention row 0:
    q = x[0]; s = X q; a = exp(s); logits = (a^T X / sum(a)) cls_w^T + cls_b
This turns ~275 GFLOP of full attention into an embedding gather plus
~4 MFLOP per batch element.

Sharding: data-parallel over batch. 8 cores x 4 batch elements; emb_table
and classifier weights replicated; no collectives.
"""

import numpy as np

import bass_rust

import concourse.bass as bass
import concourse.mybir as mybir
import concourse.tile as tile
from concourse.bass_utils import run_bass_kernel_spmd


def _split_multiwaits(nc: bass.Bass) -> None:
    """Workaround for the walrus build in this container, which rejects
    instructions carrying more than one sync-wait command ("Too many sync
    wait commands" / "ISA wrong length" in CoreV3GenImpl setupSyncWait).

    Moves each instruction's sync waits onto dedicated single-wait NOPs
    inserted right before it on the same engine stream (bass_nofuse so
    walrus's nop-fusion can't merge them back). Semantics are identical:
    the engine's sequencer blocks on the NOPs' waits, then issues the
    now-waitless instruction. Sem updates stay on the original."""
    counter = 0
    fn = nc.m.functions[0]
    for bb in fn.blocks:
        insts = bb.instructions
        new_list = []
        changed = False
        for inst in insts:
            si = inst.sync_info
            waits = list(si.on_wait) if si is not None else []
            if len(waits) > 1:
                # keep the first wait inline (walrus accepts one); move
                # only the extras onto NOPs
                for w in waits[1:]:
                    counter += 1
                    new_list.append(
                        mybir.InstNoOp(
                            name=f"waitnop-{counter}",
                            engine=inst.engine,
                            ins=[],
                            outs=[],
                            bass_nofuse=True,
                            sync_info=bass_rust.SyncInfo(on_wait=[w], on_update=[]),
                        )
                    )
                inst.sync_info = bass_rust.SyncInfo(
                    on_wait=[waits[0]], on_update=list(si.on_update)
                )
                changed = True
            new_list.append(inst)
        if changed:
            bb.instructions = new_list

B, S, D, V, C = 32, 2048, 512, 32000, 2
N_CORES = 8
BPC = B // N_CORES          # batch elements per core
SBLK = S // 128             # 16 free-dim blocks of gathered tokens

F32 = mybir.dt.float32
BF16 = mybir.dt.bfloat16
I32 = mybir.dt.int32

_CACHE: dict = {}


def _build_nc(split: bool = True) -> bass.Bass:
    nc = bass.Bass()
    emb_d = nc.dram_tensor("emb", [V, D], BF16, kind="ExternalInput")
    tok_d = nc.dram_tensor("tok", [128, BPC * SBLK], I32, kind="ExternalInput")
    cw_d = nc.dram_tensor("cls_w", [C, D], BF16, kind="ExternalInput")
    cb_d = nc.dram_tensor("cls_b", [C, 1], F32, kind="ExternalInput")
    out_d = nc.dram_tensor("out", [BPC, C], F32, kind="ExternalOutput")

    mult = mybir.AluOpType.mult
    add = mybir.AluOpType.add
    EXP = mybir.ActivationFunctionType.Exp

    with tile.TileContext(nc) as tc:
        with (
            tc.tile_pool(name="const", bufs=1) as constp,
            tc.tile_pool(name="xp", bufs=BPC) as xp,
            tc.tile_pool(name="sp", bufs=2) as sp,
            tc.tile_pool(name="jp", bufs=2) as jp,
            tc.tile_pool(name="tp", bufs=4) as tp,
            tc.tile_pool(name="ps", bufs=2, space="PSUM") as pp,
        ):
            idx0 = constp.tile([128, 1], I32)
            nc.sync.dma_start(idx0[:, :], tok_d[:, 0:1])
            idx = constp.tile([128, BPC, SBLK], I32)
            nc.sync.dma_start(idx[:, :, :], tok_d[:, :])
            ones1 = constp.tile([1, 128], BF16)
            nc.vector.memset(ones1[:], 1.0)
            ones128 = constp.tile([128, 1], F32)
            nc.vector.memset(ones128[:], 1.0)
            cw = constp.tile([C, D], BF16)
            nc.sync.dma_start(cw[:], cw_d[:, :])
            cb = constp.tile([C, 1], F32)
            nc.sync.dma_start(cb[:], cb_d[:, :])
            for b in range(BPC):
                # Gather the 2048 embedding rows for this batch element.
                # Token t lands on partition t%128, free block t//128; one
                # indirect DMA per 128-token block (one index per partition).
                x = xp.tile([128, SBLK, D], BF16, tag="x")
                for j in range(SBLK):
                    iap = (idx0[:, 0:1] if (b == 0 and j == 0)
                           else idx[:, b, j : j + 1])
                    nc.gpsimd.indirect_dma_start(
                        out=x[:, j, :],
                        out_offset=None,
                        in_=emb_d[:, :],
                        in_offset=bass.IndirectOffsetOnAxis(ap=iap, axis=0),
                    )

                # Broadcast q = x[token 0] to all 128 partitions via a K=1
                # outer-product matmul: ones[1,128]^T @ x[0:1, 0, :].
                qb = pp.tile([128, D], F32, tag="qb")
                nc.tensor.matmul(qb[:], ones1[:], x[0:1, 0, :], start=True, stop=True)
                qbs = sp.tile([128, D], BF16, tag="qbs")
                nc.scalar.copy(qbs[:], qb[:])

                # Scores s[t] = <x_t, q>: fused multiply+reduce per block
                # ((x*1) * q with accum_out = row sums).
                s = sp.tile([128, SBLK], F32, tag="s")
                for j in range(SBLK):
                    junk = jp.tile([128, D], BF16, tag="junk")
                    nc.vector.scalar_tensor_tensor(
                        out=junk[:],
                        in0=x[:, j, :],
                        scalar=1.0,
                        in1=qbs[:],
                        op0=mult,
                        op1=mult,
                        accum_out=s[:, j : j + 1],
                    )

                # a = exp(s) (scores are O(0.2): no max subtraction needed),
                # with fused per-partition row sums for the softmax denom.
                # Split into groups of 4 blocks so the pooled chain becomes
                # ready incrementally (tracks the gather stream, keeps PE
                # warm, and shrinks the end-of-kernel tail to one group).
                EG = 16
                e = sp.tile([128, SBLK], BF16, tag="e")
                zcols = sp.tile([128, EG], F32, tag="zcols")
                for g in range(EG):
                    lo, hi = g * (SBLK // EG), (g + 1) * (SBLK // EG)
                    nc.scalar.activation(
                        e[:, lo:hi], s[:, lo:hi], EXP,
                        accum_out=zcols[:, g : g + 1],
                    )

                # pooled = a^T X (unnormalized), accumulated over blocks,
                # replicated to C partitions via lhsT free-dim broadcast.
                pooled = pp.tile([C, D], F32, tag="pooled")
                for j in range(SBLK):
                    nc.tensor.matmul(
                        pooled[:],
                        e[:, j : j + 1].broadcast_to([128, C]),
                        x[:, j, :],
                        start=(j == 0),
                        stop=(j == SBLK - 1),
                    )

                # Z = sum over partitions of all zcol groups, C-replicated.
                zps = pp.tile([C, 1], F32, tag="z")
                for g in range(EG):
                    nc.tensor.matmul(
                        zps[:], zcols[:, g : g + 1].broadcast_to([128, C]),
                        ones128[:, :],
                        start=(g == 0), stop=(g == EG - 1),
                    )

                rz = tp.tile([C, 1], F32, tag="rz")
                nc.vector.reciprocal(rz[:], zps[:])

                # logits_c = <pooled, cls_w_c>: one fused dot over C rows,
                # reading pooled directly from PSUM (skips the sbuf copy)
                junk2 = tp.tile([C, D], BF16, tag="junk2")
                lg = tp.tile([C, 1], F32, tag="lg")
                nc.vector.scalar_tensor_tensor(
                    out=junk2[:],
                    in0=pooled[:],
                    scalar=1.0,
                    in1=cw[:],
                    op0=mult,
                    op1=mult,
                    accum_out=lg[:],
                )

                # out = lg / Z + cls_b
                ob = tp.tile([C, 1], F32, tag="ob")
                nc.vector.scalar_tensor_tensor(
                    ob[:], lg[:], rz[:], cb[:], op0=mult, op1=add
                )
                nc.sync.dma_start(out_d[b, :].unsqueeze(1), ob[:, :])

    nc.finalize()
    if split:
        _split_multiwaits(nc)
    return nc


def _wrap_tokens(tokens_row: np.ndarray) -> np.ndarray:
    """[S] int tokens -> [128, SBLK] int32; token t at [t%128, t//128]."""
    return np.ascontiguousarray(tokens_row.reshape(SBLK, 128).T.astype(np.int32))


def get_nc() -> bass.Bass:
    if "nc" not in _CACHE:
        _CACHE["nc"] = _build_nc()
    return _CACHE["nc"]


def make_in_maps(tokens, emb_table, cls_w, cls_b):
    import ml_dtypes

    tokens = np.asarray(tokens)
    emb = np.ascontiguousarray(
        np.asarray(emb_table, dtype=np.float32).astype(ml_dtypes.bfloat16)
    )
    cw = np.ascontiguousarray(
        np.asarray(cls_w, dtype=np.float32).astype(ml_dtypes.bfloat16)
    )
    cb = np.ascontiguousarray(np.asarray(cls_b, dtype=np.float32).reshape(C, 1))
    in_maps = []
    for core in range(N_CORES):
        idx_flat = np.concatenate(
            [_wrap_tokens(tokens[core * BPC + b]) for b in range(BPC)], axis=1
        )  # [128, BPC*SBLK]
        in_maps.append(
            {
                "emb": emb,
                "tok": np.ascontiguousarray(idx_flat),
                "cls_w": cw,
                "cls_b": cb,
            }
        )
    return in_maps


def kernel(tokens, emb_table, cls_w, cls_b) -> np.ndarray:
    nc = get_nc()
    in_maps = make_in_maps(tokens, emb_table, cls_w, cls_b)
    res = run_bass_kernel_spmd(nc, in_maps, core_ids=list(range(N_CORES)))
    outs = [res.results[c]["out"] for c in range(N_CORES)]
    return np.concatenate(outs, axis=0).astype(np.float32)



# revision 56
# speedup vs baseline: 1.0106x; 1.0035x over previous
"""Trainium2 Bass kernel for nn_AttentionModel (sparse_attention).

Reference computation:
    x = emb_table[tokens]                  # [B,S,D]
    scores = x @ x^T per batch             # [B,S,S]
    out = softmax(scores) @ x              # [B,S,D]
    logits = out[:, 0, :] @ cls_w.T + cls_b

Only row 0 of the attention output is used, so per batch element we only
need attention row 0:
    q = x[0]; s = X q; a = exp(s); logits = (a^T X / sum(a)) cls_w^T + cls_b
This turns ~275 GFLOP of full attention into an embedding gather plus
~4 MFLOP per batch element.

Sharding: data-parallel over batch. 8 cores x 4 batch elements; emb_table
and classifier weights replicated; no collectives.
"""

import numpy as np

import bass_rust

import concourse.bass as bass
import concourse.mybir as mybir
import concourse.tile as tile
from concourse.bass_utils import run_bass_kernel_spmd


def _split_multiwaits(nc: bass.Bass) -> None:
    """Workaround for the walrus build in this container, which rejects
    instructions carrying more than one sync-wait command ("Too many sync
    wait commands" / "ISA wrong length" in CoreV3GenImpl setupSyncWait).

    Moves each instruction's sync waits onto dedicated single-wait NOPs
    inserted right before it on the same engine stream (bass_nofuse so
    walrus's nop-fusion can't merge them back). Semantics are identical:
    the engine's sequencer blocks on the NOPs' waits, then issues the
    now-waitless instruction. Sem updates stay on the original."""
    counter = 0
    fn = nc.m.functions[0]
    for bb in fn.blocks:
        insts = bb.instructions
        new_list = []
        changed = False
        for inst in insts:
            si = inst.sync_info
            waits = list(si.on_wait) if si is not None else []
            if len(waits) > 1:
                # keep the first wait inline (walrus accepts one); move
                # only the extras onto NOPs
                for w in waits[:-1]:
                    counter += 1
                    new_list.append(
                        mybir.InstNoOp(
                            name=f"waitnop-{counter}",
                            engine=inst.engine,
                            ins=[],
                            outs=[],
                            bass_nofuse=True,
                            sync_info=bass_rust.SyncInfo(on_wait=[w], on_update=[]),
                        )
                    )
                inst.sync_info = bass_rust.SyncInfo(
                    on_wait=[waits[-1]], on_update=list(si.on_update)
                )
                changed = True
            new_list.append(inst)
        if changed:
            bb.instructions = new_list

B, S, D, V, C = 32, 2048, 512, 32000, 2
N_CORES = 8
BPC = B // N_CORES          # batch elements per core
SBLK = S // 128             # 16 free-dim blocks of gathered tokens

F32 = mybir.dt.float32
BF16 = mybir.dt.bfloat16
I32 = mybir.dt.int32

_CACHE: dict = {}


def _build_nc(split: bool = True) -> bass.Bass:
    nc = bass.Bass()
    emb_d = nc.dram_tensor("emb", [V, D], BF16, kind="ExternalInput")
    tok_d = nc.dram_tensor("tok", [128, BPC * SBLK], I32, kind="ExternalInput")
    cw_d = nc.dram_tensor("cls_w", [C, D], BF16, kind="ExternalInput")
    cb_d = nc.dram_tensor("cls_b", [C, 1], F32, kind="ExternalInput")
    out_d = nc.dram_tensor("out", [BPC, C], F32, kind="ExternalOutput")

    mult = mybir.AluOpType.mult
    add = mybir.AluOpType.add
    EXP = mybir.ActivationFunctionType.Exp

    with tile.TileContext(nc) as tc:
        with (
            tc.tile_pool(name="const", bufs=1) as constp,
            tc.tile_pool(name="xp", bufs=BPC) as xp,
            tc.tile_pool(name="sp", bufs=4) as sp,
            tc.tile_pool(name="jp", bufs=2) as jp,
            tc.tile_pool(name="tp", bufs=4) as tp,
            tc.tile_pool(name="ps", bufs=2, space="PSUM") as pp,
        ):
            idx0 = constp.tile([128, 1], I32)
            nc.sync.dma_start(idx0[:, :], tok_d[:, 0:1])
            idx = constp.tile([128, BPC, SBLK], I32)
            nc.sync.dma_start(idx[:, :, :], tok_d[:, :])
            ones1 = constp.tile([1, 128], BF16)
            nc.vector.memset(ones1[:], 1.0)
            ones128 = constp.tile([128, 1], F32)
            nc.vector.memset(ones128[:], 1.0)
            cw = constp.tile([C, D], BF16)
            nc.sync.dma_start(cw[:], cw_d[:, :])
            cb = constp.tile([C, 1], F32)
            nc.sync.dma_start(cb[:], cb_d[:, :])
            for b in range(BPC):
                # Gather the 2048 embedding rows for this batch element.
                # Token t lands on partition t%128, free block t//128; one
                # indirect DMA per 128-token block (one index per partition).
                x = xp.tile([128, SBLK, D], BF16, tag="x")
                for j in range(SBLK):
                    iap = (idx0[:, 0:1] if (b == 0 and j == 0)
                           else idx[:, b, j : j + 1])
                    nc.gpsimd.indirect_dma_start(
                        out=x[:, j, :],
                        out_offset=None,
                        in_=emb_d[:, :],
                        in_offset=bass.IndirectOffsetOnAxis(ap=iap, axis=0),
                    )

                # Broadcast q = x[token 0] to all 128 partitions via a K=1
                # outer-product matmul: ones[1,128]^T @ x[0:1, 0, :].
                qb = pp.tile([128, D], F32, tag="qb")
                nc.tensor.matmul(qb[:], ones1[:], x[0:1, 0, :], start=True, stop=True)
                qbs = sp.tile([128, D], BF16, tag="qbs")
                nc.scalar.copy(qbs[:], qb[:])

                # Scores s[t] = <x_t, q>: fused multiply+reduce per block
                # ((x*1) * q with accum_out = row sums).
                s = sp.tile([128, SBLK], F32, tag="s")
                for j in range(SBLK):
                    junk = jp.tile([128, D], BF16, tag="junk")
                    nc.vector.scalar_tensor_tensor(
                        out=junk[:],
                        in0=x[:, j, :],
                        scalar=1.0,
                        in1=qbs[:],
                        op0=mult,
                        op1=mult,
                        accum_out=s[:, j : j + 1],
                    )

                # a = exp(s) (scores are O(0.2): no max subtraction needed),
                # with fused per-partition row sums for the softmax denom.
                # Split into groups of 4 blocks so the pooled chain becomes
                # ready incrementally (tracks the gather stream, keeps PE
                # warm, and shrinks the end-of-kernel tail to one group).
                EG = 16
                e = sp.tile([128, SBLK], BF16, tag="e")
                zcols = sp.tile([128, EG], F32, tag="zcols")
                for g in range(EG):
                    lo, hi = g * (SBLK // EG), (g + 1) * (SBLK // EG)
                    nc.scalar.activation(
                        e[:, lo:hi], s[:, lo:hi], EXP,
                        accum_out=zcols[:, g : g + 1],
                    )

                # pooled = a^T X (unnormalized), accumulated over blocks,
                # replicated to C partitions via lhsT free-dim broadcast.
                pooled = pp.tile([C, D], F32, tag="pooled")
                for j in range(SBLK):
                    nc.tensor.matmul(
                        pooled[:],
                        e[:, j : j + 1].broadcast_to([128, C]),
                        x[:, j, :],
                        start=(j == 0),
                        stop=(j == SBLK - 1),
                    )

                # Z = sum over partitions of all zcol groups, C-replicated.
                zps = pp.tile([C, 1], F32, tag="z")
                for g in range(EG):
                    nc.tensor.matmul(
                        zps[:], zcols[:, g : g + 1].broadcast_to([128, C]),
                        ones128[:, :],
                        start=(g == 0), stop=(g == EG - 1),
                    )

                rz = tp.tile([C, 1], F32, tag="rz")
                nc.vector.reciprocal(rz[:], zps[:])

                # logits_c = <pooled, cls_w_c>: one fused dot over C rows,
                # reading pooled directly from PSUM (skips the sbuf copy)
                junk2 = tp.tile([C, D], BF16, tag="junk2")
                lg = tp.tile([C, 1], F32, tag="lg")
                nc.vector.scalar_tensor_tensor(
                    out=junk2[:],
                    in0=pooled[:],
                    scalar=1.0,
                    in1=cw[:],
                    op0=mult,
                    op1=mult,
                    accum_out=lg[:],
                )

                # out = lg / Z + cls_b
                ob = tp.tile([C, 1], F32, tag="ob")
                nc.vector.scalar_tensor_tensor(
                    ob[:], lg[:], rz[:], cb[:], op0=mult, op1=add
                )
                nc.sync.dma_start(out_d[b, :].unsqueeze(1), ob[:, :])

    nc.finalize()
    if split:
        _split_multiwaits(nc)
    return nc


def _wrap_tokens(tokens_row: np.ndarray) -> np.ndarray:
    """[S] int tokens -> [128, SBLK] int32; token t at [t%128, t//128]."""
    return np.ascontiguousarray(tokens_row.reshape(SBLK, 128).T.astype(np.int32))


def get_nc() -> bass.Bass:
    if "nc" not in _CACHE:
        _CACHE["nc"] = _build_nc()
    return _CACHE["nc"]


def make_in_maps(tokens, emb_table, cls_w, cls_b):
    import ml_dtypes

    tokens = np.asarray(tokens)
    emb = np.ascontiguousarray(
        np.asarray(emb_table, dtype=np.float32).astype(ml_dtypes.bfloat16)
    )
    cw = np.ascontiguousarray(
        np.asarray(cls_w, dtype=np.float32).astype(ml_dtypes.bfloat16)
    )
    cb = np.ascontiguousarray(np.asarray(cls_b, dtype=np.float32).reshape(C, 1))
    in_maps = []
    for core in range(N_CORES):
        idx_flat = np.concatenate(
            [_wrap_tokens(tokens[core * BPC + b]) for b in range(BPC)], axis=1
        )  # [128, BPC*SBLK]
        in_maps.append(
            {
                "emb": emb,
                "tok": np.ascontiguousarray(idx_flat),
                "cls_w": cw,
                "cls_b": cb,
            }
        )
    return in_maps


def kernel(tokens, emb_table, cls_w, cls_b) -> np.ndarray:
    nc = get_nc()
    in_maps = make_in_maps(tokens, emb_table, cls_w, cls_b)
    res = run_bass_kernel_spmd(nc, in_maps, core_ids=list(range(N_CORES)))
    outs = [res.results[c]["out"] for c in range(N_CORES)]
    return np.concatenate(outs, axis=0).astype(np.float32)

